# revision 1
# baseline (speedup 1.0000x reference)
"""ChebNet (K=5, 3 layers) on 8 trn2 NeuronCores.

Sharding: dst-nodes across cores (6250/core, padded to 6272 = 49*128 slots,
degree-sorted within core so per-128-slot-tile max in-degree ~ mean).
Propagation h' = L_hat h is computed as:
    g = dinv * h                           (per-node scale, done where h is made)
    Ptilde[slot] = sum_t g[src(t, slot)]   (indirect gather + identity-matmul
                                            accumulate into PSUM)
    (L_hat h)[slot] = -dinv[slot] * Ptilde (per-partition scale in the
                                            Chebyshev recurrence)
Features are replicated via AllGather of the per-core g slices after each hop.
"""
import os
import numpy as np

N_NODES = 50000
N_EDGES = 800000
N_GRAPHS = 64
KCH = 5
C = 128
C_OUT = 16
NCORES = 8
NPC = N_NODES // NCORES          # 6250 real nodes per core
D_TILES = 49
SPC = D_TILES * 128              # 6272 slots per core
NSLOT = NCORES * SPC             # 50176 rows in slot space
P = 128
TCALL = 24                       # edge-tiles per gather call
ZERO_ROW = SPC - 1               # core0's last pad slot: always zero

_G_BF16 = os.environ.get("GNN_G_BF16", "1") == "1"


# ----------------------------------------------------------------------------
# host-side graph preprocessing (index/layout only + norm scalars)
# ----------------------------------------------------------------------------

def prep(x, src, dst, batch):
    src = np.asarray(src).astype(np.int64)
    dst = np.asarray(dst).astype(np.int64)
    x = np.asarray(x, dtype=np.float32)
    batch = np.asarray(batch).astype(np.int64)

    deg = np.bincount(src, minlength=N_NODES).astype(np.float32)
    dinv = np.where(
        deg > 0,
        (1.0 / np.sqrt(np.maximum(deg, 1.0))).astype(np.float32),
        np.float32(0.0),
    ).astype(np.float32)

    indeg = np.bincount(dst, minlength=N_NODES)

    # slot mapping: per core, sort its nodes by in-degree desc
    node_slotrow = np.zeros(N_NODES, np.int64)
    slot_node = np.full((NCORES, SPC), -1, np.int64)
    for c in range(NCORES):
        nodes = np.arange(c * NPC, (c + 1) * NPC)
        order = np.argsort(-indeg[nodes], kind="stable")
        sn = nodes[order]
        slot_node[c, :NPC] = sn
        node_slotrow[sn] = c * SPC + np.arange(NPC)

    # per-dst-tile column counts, maxed across cores (SPMD-uniform program)
    T = np.zeros(D_TILES, np.int64)
    for c in range(NCORES):
        for d in range(D_TILES):
            sl = slot_node[c, d * 128:(d + 1) * 128]
            sl = sl[sl >= 0]
            if sl.size:
                T[d] = max(T[d], int(indeg[sl].max()))
    T = np.maximum(T, 1)
    col_base = np.concatenate([[0], np.cumsum(T)])
    tot_tiles = int(T.sum())
    col2d = np.zeros(tot_tiles, np.int64)
    for d in range(D_TILES):
        col2d[col_base[d]:col_base[d + 1]] = d

    # edge grid: for each core, idx[lane, col] = slot row of source
    order = np.argsort(dst, kind="stable")
    ds = dst[order]
    ss = src[order]
    first = np.flatnonzero(np.r_[True, ds[1:] != ds[:-1]])
    sizes = np.diff(np.r_[first, N_EDGES])
    rank = np.arange(N_EDGES) - np.repeat(first, sizes)

    srow = node_slotrow[ds]
    core = srow // SPC
    slocal = srow % SPC
    dtile = slocal // 128
    lane = slocal % 128
    col = col_base[dtile] + rank

    idx_grids = np.full((NCORES, P, tot_tiles), ZERO_ROW, np.int32)
    idx_grids[core, lane, col] = node_slotrow[ss].astype(np.int32)

    # per-core [128, 49] tables (lane p, tile d -> slot d*128+p)
    dinv_slot = np.zeros((NCORES, P, D_TILES), np.float32)
    batid_slot = np.full((NCORES, P, D_TILES), 64.0, np.float32)
    x_local = np.zeros((NCORES, SPC, C), np.float32)
    for c in range(NCORES):
        sn = slot_node[c]
        valid = sn >= 0
        dv = np.where(valid, dinv[np.maximum(sn, 0)], 0.0).astype(np.float32)
        bt = np.where(valid, batch[np.maximum(sn, 0)], 64).astype(np.float32)
        dinv_slot[c] = dv.reshape(D_TILES, P).T
        batid_slot[c] = bt.reshape(D_TILES, P).T
        x_local[c][valid] = x[sn[valid]]

    # initial gather source: g0[slotrow] = dinv[n] * x[n]
    g0 = np.zeros((NSLOT, C), np.float32)
    for c in range(NCORES):
        sn = slot_node[c]
        valid = sn >= 0
        g0[c * SPC:(c + 1) * SPC][valid] = (
            x[sn[valid]] * dinv[sn[valid]][:, None]
        )

    cnts = np.bincount(batch, minlength=N_GRAPHS).astype(np.float32)
    inv_cnt = (1.0 / np.maximum(cnts, 1.0)).astype(np.float32)

    iota64 = np.tile(np.arange(64, dtype=np.float32), (P, 1))
    ident = np.eye(P, dtype=np.float32)

    return dict(
        T=T, col_base=col_base, tot_tiles=tot_tiles, col2d=col2d,
        idx_grids=idx_grids, dinv_slot=dinv_slot, batid_slot=batid_slot,
        x_local=x_local, g0=g0, inv_cnt=inv_cnt, iota64=iota64, ident=ident,
        slot_node=slot_node, node_slotrow=node_slotrow, dinv=dinv,
    )


# ----------------------------------------------------------------------------
# bass program
# ----------------------------------------------------------------------------

def build(col2d, tot_tiles, g_bf16):
    import concourse.bacc as bacc
    import concourse.bass as bass
    import concourse.mybir as mybir
    import concourse.tile as tile

    f32 = mybir.dt.float32
    g_dt = mybir.dt.bfloat16 if g_bf16 else f32
    AF = mybir.ActivationFunctionType
    OP = mybir.AluOpType

    nc = bacc.Bacc("TRN2", target_bir_lowering=False, debug=False,
                   num_devices=NCORES)

    g0_in = nc.dram_tensor("g0", [NSLOT, C], g_dt, kind="ExternalInput")
    xl_in = nc.dram_tensor("x_local", [SPC, C], f32, kind="ExternalInput")
    idx_in = nc.dram_tensor("idxg", [P, tot_tiles], mybir.dt.int32, kind="ExternalInput")
    dinv_in = nc.dram_tensor("dinv", [P, D_TILES], f32, kind="ExternalInput")
    bat_in = nc.dram_tensor("batid", [P, D_TILES], f32, kind="ExternalInput")
    iota_in = nc.dram_tensor("iota64", [P, 64], f32, kind="ExternalInput")
    id_in = nc.dram_tensor("ident", [P, P], f32, kind="ExternalInput")
    w1_in = nc.dram_tensor("W1", [KCH, C, C], f32, kind="ExternalInput")
    w2_in = nc.dram_tensor("W2", [KCH, C, C], f32, kind="ExternalInput")
    w3_in = nc.dram_tensor("W3", [KCH, C, C_OUT], f32, kind="ExternalInput")
    b1_in = nc.dram_tensor("b1", [C, 1], f32, kind="ExternalInput")
    b2_in = nc.dram_tensor("b2", [C, 1], f32, kind="ExternalInput")
    b3r_in = nc.dram_tensor("b3row", [P, C_OUT], f32, kind="ExternalInput")
    ic_in = nc.dram_tensor("inv_cnt", [N_GRAPHS, 1], f32, kind="ExternalInput")
    out_t = nc.dram_tensor("out", [N_GRAPHS, C_OUT], f32, kind="ExternalOutput")
    dbg_t = nc.dram_tensor("dbg", [P, D_TILES * C], f32, kind="ExternalOutput")
    dbg_tap = int(os.environ.get("GNN_DEBUG_TAP", "-1"))

    rg = [list(range(NCORES))]

    with tile.TileContext(nc) as tc:
        with (
            tc.tile_pool(name="const", bufs=1) as cst,
            tc.tile_pool(name="tx", bufs=1) as txp,
            tc.tile_pool(name="gath", bufs=3) as gap,
            tc.tile_pool(name="stg", bufs=4) as stg,
            tc.tile_pool(name="psA", bufs=3, space="PSUM") as psA,
            tc.tile_pool(name="psB", bufs=1, space="PSUM") as psB,
            tc.tile_pool(name="dram", bufs=1, space="DRAM") as drp,
        ):
            n_ag = 11
            gbufs = [drp.tile([NSLOT, C], g_dt, addr_space="Shared",
                              name=f"gbuf{i}") for i in range(n_ag)]
            ag_in = drp.tile([SPC, C], g_dt, name="ag_in")
            cc_in = drp.tile([N_GRAPHS, C_OUT], f32, name="cc_in")
            cc_out = drp.tile([N_GRAPHS, C_OUT], f32, addr_space="Shared", name="cc_out")

            idx_sb = cst.tile([P, tot_tiles], mybir.dt.int32, name="idx_sb")
            nc.sync.dma_start(idx_sb[:], idx_in[:])
            dinv = cst.tile([P, D_TILES], f32, name="dinv_sb")
            nc.sync.dma_start(dinv[:], dinv_in[:])
            mdinv = cst.tile([P, D_TILES], f32, name="mdinv_sb")
            nc.vector.tensor_scalar_mul(mdinv[:], dinv[:], -1.0)
            m2dinv = cst.tile([P, D_TILES], f32, name="m2dinv_sb")
            nc.vector.tensor_scalar_mul(m2dinv[:], dinv[:], -2.0)
            batid = cst.tile([P, D_TILES], f32, name="batid_sb")
            nc.sync.dma_start(batid[:], bat_in[:])
            iota64 = cst.tile([P, 64], f32, name="iota64_sb")
            nc.sync.dma_start(iota64[:], iota_in[:])
            identf = cst.tile([P, P], f32, name="identf_sb")
            nc.sync.dma_start(identf[:], id_in[:])
            if g_bf16:
                identg = cst.tile([P, P], g_dt, name="identg_sb")
                nc.vector.tensor_copy(identg[:], identf[:])
            else:
                identg = identf
            w1 = cst.tile([P, KCH * C], f32, name="w1_sb")
            w2 = cst.tile([P, KCH * C], f32, name="w2_sb")
            w3 = cst.tile([P, KCH * C_OUT], f32, name="w3_sb")
            for k in range(KCH):
                nc.sync.dma_start(w1[:, k * C:(k + 1) * C], w1_in[k])
                nc.sync.dma_start(w2[:, k * C:(k + 1) * C], w2_in[k])
                nc.sync.dma_start(w3[:, k * C_OUT:(k + 1) * C_OUT], w3_in[k])
            b1 = cst.tile([C, 1], f32, name="b1_sb")
            nc.sync.dma_start(b1[:], b1_in[:])
            b2 = cst.tile([C, 1], f32, name="b2_sb")
            nc.sync.dma_start(b2[:], b2_in[:])
            b3row = cst.tile([P, C_OUT], f32, name="b3row_sb")
            nc.sync.dma_start(b3row[:], b3r_in[:])
            invc = cst.tile([N_GRAPHS, 1], f32, name="invc_sb")
            nc.sync.dma_start(invc[:], ic_in[:])

            # node-major Chebyshev buffers [128 lanes, 49*128] (lane, d*128+f)
            tx = [txp.tile([P, D_TILES * C], f32, name=f"tx{k}_sb")
                  for k in range(KCH)]
            for d in range(D_TILES):
                nc.sync.dma_start(tx[0][:, d * C:(d + 1) * C],
                                  xl_in[d * P:(d + 1) * P, :])

            def do_prop(p_idx, k, src_dram, tail_fn=None):
                acc = None
                cur_d = -1
                gt = None
                for t in range(tot_tiles):
                    gt = gap.tile([P, C], g_dt, tag="gt", bufs=8)
                    nc.gpsimd.indirect_dma_start(
                        out=gt[:],
                        out_offset=None,
                        in_=src_dram[:],
                        in_offset=bass.IndirectOffsetOnAxis(
                            ap=idx_sb[:, t:t + 1], axis=0),
                    )
                    if dbg_tap == 100 + p_idx and t == 0:
                        nc.sync.dma_start(dbg_t[:, :C], gt[:])
                    j = 0
                    d = int(col2d[t])
                    if d != cur_d:
                        acc = psA.tile([P, C], f32, tag="acc", bufs=3)
                        cur_d = d
                    first = (t == 0) or (int(col2d[t - 1]) != d)
                    last = (t == tot_tiles - 1) or (int(col2d[t + 1]) != d)
                    nc.tensor.matmul(acc[:], identg[:], gt[:],
                                     start=first, stop=last)
                    if last:
                        dc = slice(d * C, (d + 1) * C)
                        if k == 1:
                            nc.vector.tensor_scalar(
                                tx[1][:, dc], acc[:], mdinv[:, d:d + 1], None,
                                OP.mult)
                        else:
                            tmp = stg.tile([P, C], f32, tag="rtmp")
                            nc.vector.tensor_scalar(
                                tmp[:], acc[:], m2dinv[:, d:d + 1], None,
                                OP.mult)
                            nc.vector.tensor_tensor(
                                tx[k][:, dc], tmp[:], tx[k - 2][:, dc],
                                OP.subtract)
                        if k <= 3:
                            gs = stg.tile([P, C], g_dt, tag="gs")
                            nc.vector.tensor_scalar(
                                gs[:], tx[k][:, dc], dinv[:, d:d + 1], None,
                                OP.mult)
                            nc.sync.dma_start(
                                ag_in[d * P:(d + 1) * P, :], gs[:])
                        if tail_fn is not None:
                            tail_fn(d)

            def do_ag(dst_buf):
                nc.gpsimd.collective_compute(
                    "AllGather", mybir.AluOpType.bypass,
                    replica_groups=rg,
                    ins=[ag_in.opt()],
                    outs=[dst_buf.opt()],
                )

            p_idx = 0
            ag_i = 0
            cur_src = g0_in
            pool_ps = None
            for layer in range(3):
                wsb = (w1, w2, w3)[layer]
                for k in range(1, KCH):
                    do_prop(p_idx, k, cur_src)
                    if dbg_tap == p_idx:
                        nc.sync.dma_start(dbg_t[:], tx[k][:])
                    if k <= 3:
                        do_ag(gbufs[ag_i])
                        cur_src = gbufs[ag_i]
                        ag_i += 1
                    p_idx += 1

                # layer end: out = sum_k Txk @ W[k] (+bias, relu)
                for d in range(D_TILES):
                    dc = slice(d * C, (d + 1) * C)
                    if layer < 2:
                        ops = psB.tile([C, C], f32, tag="wout", bufs=1)
                        for k in range(KCH):
                            tp = psB.tile([P, C], f32, tag="tp", bufs=2)
                            nc.tensor.transpose(tp[:], tx[k][:, dc], identf[:])
                            st = stg.tile([P, C], f32, tag="stgT")
                            nc.vector.tensor_copy(st[:], tp[:])
                            nc.tensor.matmul(
                                ops[:], wsb[:, k * C:(k + 1) * C], st[:],
                                start=(k == 0), stop=(k == KCH - 1))
                        hT = stg.tile([P, C], f32, tag="hT")
                        bsb = b1 if layer == 0 else b2
                        nc.scalar.activation(hT[:], ops[:], AF.Relu, bias=bsb[:])
                        nmp = psB.tile([P, C], f32, tag="nmp", bufs=1)
                        nc.tensor.transpose(nmp[:], hT[:], identf[:])
                        nc.vector.tensor_copy(tx[0][:, dc], nmp[:])
                        gs = stg.tile([P, C], g_dt, tag="gs2")
                        nc.vector.tensor_scalar(
                            gs[:], tx[0][:, dc], dinv[:, d:d + 1], None,
                            OP.mult)
                        nc.sync.dma_start(ag_in[d * P:(d + 1) * P, :], gs[:])
                    else:
                        nm3 = psB.tile([P, C_OUT], f32, tag="wout", bufs=1)
                        for k in range(KCH):
                            tp = psB.tile([P, C], f32, tag="tp", bufs=2)
                            nc.tensor.transpose(tp[:], tx[k][:, dc], identf[:])
                            st = stg.tile([P, C], f32, tag="stgT")
                            nc.vector.tensor_copy(st[:], tp[:])
                            nc.tensor.matmul(
                                nm3[:], st[:], w3[:, k * C_OUT:(k + 1) * C_OUT],
                                start=(k == 0), stop=(k == KCH - 1))
                        h3 = stg.tile([P, C_OUT], f32, tag="h3nm")
                        nc.vector.tensor_tensor(h3[:], nm3[:], b3row[:], OP.add)
                        B = stg.tile([P, 64], f32, tag="Bt")
                        nc.vector.tensor_scalar(
                            B[:], iota64[:], batid[:, d:d + 1], None,
                            OP.is_equal)
                        if pool_ps is None:
                            pool_ps = psB.tile([N_GRAPHS, C_OUT], f32,
                                               tag="pool", bufs=1)
                        nc.tensor.matmul(pool_ps[:], B[:], h3[:],
                                         start=(d == 0), stop=(d == D_TILES - 1))
                if layer < 2:
                    do_ag(gbufs[ag_i])
                    cur_src = gbufs[ag_i]
                    ag_i += 1

            # pooling: partial sums -> AllReduce -> mean -> log_softmax
            pool_sb = stg.tile([N_GRAPHS, C_OUT], f32, name="pool_sb")
            nc.vector.tensor_copy(pool_sb[:], pool_ps[:])
            nc.sync.dma_start(cc_in[:], pool_sb[:])
            nc.gpsimd.collective_compute(
                "AllReduce", mybir.AluOpType.add, replica_groups=rg,
                ins=[cc_in.opt()], outs=[cc_out.opt()])
            pooled = stg.tile([N_GRAPHS, C_OUT], f32, name="pooled")
            nc.sync.dma_start(pooled[:], cc_out[:])
            pmean = stg.tile([N_GRAPHS, C_OUT], f32, name="pmean")
            nc.vector.tensor_scalar(pmean[:], pooled[:], invc[:], None, OP.mult)
            mx = stg.tile([N_GRAPHS, 1], f32, name="mx")
            nc.vector.tensor_reduce(mx[:], pmean[:], mybir.AxisListType.X, OP.max)
            z = stg.tile([N_GRAPHS, C_OUT], f32, name="zt")
            nc.vector.tensor_scalar(z[:], pmean[:], mx[:], None, OP.subtract)
            ez = stg.tile([N_GRAPHS, C_OUT], f32, name="ez")
            nc.scalar.activation(ez[:], z[:], AF.Exp)
            sm = stg.tile([N_GRAPHS, 1], f32, name="sm")
            nc.vector.tensor_reduce(sm[:], ez[:], mybir.AxisListType.X, OP.add)
            lg = stg.tile([N_GRAPHS, 1], f32, name="lg")
            nc.scalar.activation(lg[:], sm[:], AF.Ln)
            res = stg.tile([N_GRAPHS, C_OUT], f32, name="res")
            nc.vector.tensor_scalar(res[:], z[:], lg[:], None, OP.subtract)
            nc.sync.dma_start(out_t[:], res[:])

    nc.compile()
    return nc


# ----------------------------------------------------------------------------
# entry point
# ----------------------------------------------------------------------------

_CACHE = {}


def _run(inputs, trace=False):
    from concourse.bass_utils import run_bass_kernel_spmd

    pp = prep(inputs["x"], inputs["src"], inputs["dst"], inputs["batch"])
    key = (int(pp["tot_tiles"]), tuple(pp["col2d"][::17]), _G_BF16)
    if key not in _CACHE:
        _CACHE[key] = build(pp["col2d"], pp["tot_tiles"], _G_BF16)
    nc = _CACHE[key]

    g0 = pp["g0"]
    if _G_BF16:
        import ml_dtypes
        g0 = g0.astype(ml_dtypes.bfloat16)

    b3row = np.tile(np.asarray(inputs["b3"], np.float32).reshape(1, C_OUT),
                    (P, 1))
    in_maps = []
    for c in range(NCORES):
        in_maps.append({
            "g0": g0,
            "x_local": pp["x_local"][c],
            "idxg": pp["idx_grids"][c],
            "dinv": pp["dinv_slot"][c],
            "batid": pp["batid_slot"][c],
            "iota64": pp["iota64"],
            "ident": pp["ident"],
            "W1": np.asarray(inputs["W1"], np.float32),
            "W2": np.asarray(inputs["W2"], np.float32),
            "W3": np.asarray(inputs["W3"], np.float32),
            "b1": np.asarray(inputs["b1"], np.float32).reshape(C, 1),
            "b2": np.asarray(inputs["b2"], np.float32).reshape(C, 1),
            "b3row": b3row,
            "inv_cnt": pp["inv_cnt"].reshape(N_GRAPHS, 1),
        })
    res = run_bass_kernel_spmd(nc, in_maps, list(range(NCORES)), trace=trace)
    return res.results[0]["out"], res


def kernel(**inputs) -> np.ndarray:
    out, _ = _run(inputs, trace=False)
    return np.asarray(out, dtype=np.float32)

